# revision 5
# baseline (speedup 1.0000x reference)
"""Trainium2 Bass kernel for a single AttnDecoderRNN step (GRU + attention +
vocab projection + log_softmax), sharded across 8 NeuronCores.

Sharding:
  - GRU: hidden dim sharded (128 rows/core); gates computed per-core.
  - attention: encoder sequence sharded (1024 positions/core); local softmax
    stats merged via a tiny AllGather; context via AllReduce.
  - output Linear: vocab sharded (6656 rows/core, padded), out_W pre-transposed
    on host to [vs, kc, 128, 512] tiles in bf16; logits accumulated on PE.
  - log_softmax: local max/sumexp per core, merged via AllGather of stats.

kernel(**inputs) takes FULL inputs (as produced by setup_inputs()) and returns
the FULL outputs (output, context, hidden, attn_weights) like the reference.
"""
import numpy as np

H = 1024
V = 50257
S = 8192
NC = 8

HC = H // NC          # 128  per-core hidden chunk
SC = S // NC          # 1024 per-core sequence chunk
NSL = 13              # vocab slices of 512 per core
VSP = NSL * 512       # 6656 per-core padded vocab shard
VP = VSP * NC         # 53248 padded vocab
K2 = 2 * H            # 2048 input dim of out_W / W_ih

_F32 = None
_BF16 = None
_cache = {}


def _dt():
    from concourse import mybir
    return mybir.dt.float32, mybir.dt.bfloat16


def _build():
    """Build + compile the 8-core SPMD Bass program. Cached per process."""
    if "nc" in _cache:
        return _cache["nc"]

    from concourse import bacc, mybir, tile, bass_isa

    f32 = mybir.dt.float32
    bf16 = mybir.dt.bfloat16
    AF = mybir.ActivationFunctionType
    OP = mybir.AluOpType
    RG = [list(range(NC))]

    nc = bacc.Bacc("TRN2", target_bir_lowering=False, debug=False,
                   num_devices=NC)

    # ---- per-core external inputs ----
    xin = nc.dram_tensor("xin", [K2], f32, kind="ExternalInput")         # [emb|ctx] replicated
    hin = nc.dram_tensor("hin", [H], f32, kind="ExternalInput")          # full h, replicated
    hch = nc.dram_tensor("hch", [HC], f32, kind="ExternalInput")         # own h chunk
    wgt = nc.dram_tensor("wgt", [16, 128, 384], f32, kind="ExternalInput")
    wht = nc.dram_tensor("wht", [8, 128, 384], f32, kind="ExternalInput")
    biasv = nc.dram_tensor("biasv", [128, 4], f32, kind="ExternalInput")
    attAr = nc.dram_tensor("attAr", [128, H], f32, kind="ExternalInput")  # attn_W rows shard
    encs = nc.dram_tensor("encs", [8, 128, H], f32, kind="ExternalInput")
    wtsh = nc.dram_tensor("wtsh", [NSL, 8, 128, 512], bf16, kind="ExternalInput")
    wtsc = nc.dram_tensor("wtsc", [NSL, 8, 128, 512], bf16, kind="ExternalInput")
    obv = nc.dram_tensor("obv", [VSP], f32, kind="ExternalInput")

    # ---- per-core external outputs ----
    out_part = nc.dram_tensor("out_part", [VSP], f32, kind="ExternalOutput")
    ctx_out = nc.dram_tensor("ctx_out", [H], f32, kind="ExternalOutput")
    hid_part = nc.dram_tensor("hid_part", [HC], f32, kind="ExternalOutput")
    attw_part = nc.dram_tensor("attw_part", [SC], f32, kind="ExternalOutput")

    with tile.TileContext(nc) as tc:
        with tc.tile_pool(name="const", bufs=1) as cb, \
             tc.tile_pool(name="stream", bufs=5) as stp, \
             tc.tile_pool(name="scratch", bufs=2) as scr, \
             tc.tile_pool(name="psum", bufs=1, space="PSUM") as ps, \
             tc.tile_pool(name="psuml", bufs=2, space="PSUM") as psl, \
             tc.tile_pool(name="dram", bufs=1, space="DRAM") as dr:

            # ================= GRU =================
            xt = cb.tile([128, 16], f32)
            nc.sync.dma_start(out=xt[:], in_=xin.ap().rearrange("(a b) -> b a", b=128))
            ht = cb.tile([128, 8], f32)
            nc.sync.dma_start(out=ht[:], in_=hin.ap().rearrange("(a b) -> b a", b=128))
            hcht = cb.tile([128, 1], f32)
            nc.sync.dma_start(out=hcht[:], in_=hch.ap().rearrange("(b o) -> b o", o=1))
            bias_sb = cb.tile([128, 4], f32)
            nc.sync.dma_start(out=bias_sb[:], in_=biasv[:])
            wg = cb.tile([128, 16, 384], f32)
            nc.sync.dma_start(out=wg[:], in_=wgt.ap().rearrange("a b c -> b a c"))
            wh = cb.tile([128, 8, 384], f32)
            nc.sync.dma_start(out=wh[:], in_=wht.ap().rearrange("a b c -> b a c"))

            ps_rz = ps.tile([128, 2], f32, tag="psrz")
            ps_n = ps.tile([128, 2], f32, tag="psn")
            for g in range(2):  # r, z
                for kc in range(16):
                    nc.tensor.matmul(ps_rz[:, g:g + 1],
                                     lhsT=wg[:, kc, g * 128:(g + 1) * 128],
                                     rhs=xt[:, kc:kc + 1], start=(kc == 0), stop=False)
                for kc in range(8):
                    nc.tensor.matmul(ps_rz[:, g:g + 1],
                                     lhsT=wh[:, kc, g * 128:(g + 1) * 128],
                                     rhs=ht[:, kc:kc + 1], start=False, stop=(kc == 7))
            for kc in range(16):  # gi_n
                nc.tensor.matmul(ps_n[:, 0:1], lhsT=wg[:, kc, 256:384],
                                 rhs=xt[:, kc:kc + 1], start=(kc == 0), stop=(kc == 15))
            for kc in range(8):   # gh_n
                nc.tensor.matmul(ps_n[:, 1:2], lhsT=wh[:, kc, 256:384],
                                 rhs=ht[:, kc:kc + 1], start=(kc == 0), stop=(kc == 7))

            r_g = cb.tile([128, 1], f32)
            nc.scalar.activation(r_g[:], ps_rz[:, 0:1], AF.Sigmoid, bias=bias_sb[:, 0:1])
            z_g = cb.tile([128, 1], f32)
            nc.scalar.activation(z_g[:], ps_rz[:, 1:2], AF.Sigmoid, bias=bias_sb[:, 1:2])
            tn = cb.tile([128, 1], f32)
            nc.vector.tensor_scalar_add(tn[:], ps_n[:, 1:2], bias_sb[:, 3:4])
            tn2 = cb.tile([128, 1], f32)
            nc.vector.tensor_mul(tn2[:], r_g[:], tn[:])
            t3 = cb.tile([128, 1], f32)
            nc.vector.tensor_add(t3[:], ps_n[:, 0:1], tn2[:])
            n_g = cb.tile([128, 1], f32)
            nc.scalar.activation(n_g[:], t3[:], AF.Tanh, bias=bias_sb[:, 2:3])
            d_g = cb.tile([128, 1], f32)
            nc.vector.tensor_sub(d_g[:], hcht[:], n_g[:])
            zd = cb.tile([128, 1], f32)
            nc.vector.tensor_mul(zd[:], z_g[:], d_g[:])
            hnew = cb.tile([128, 1], f32)
            nc.vector.tensor_add(hnew[:], n_g[:], zd[:])
            nc.sync.dma_start(out=hid_part.ap().rearrange("(p o) -> p o", o=1), in_=hnew[:])

            # ============ v partial (attn_W row-shard) + fused AllGather ============
            var = cb.tile([128, H], f32)
            nc.sync.dma_start(out=var[:], in_=attAr[:])
            ps_v = ps.tile([128, 8], f32, tag="psv")
            for c in range(8):
                nc.tensor.matmul(ps_v[:, c:c + 1], lhsT=var[:, c * 128:(c + 1) * 128],
                                 rhs=hnew[:], start=True, stop=True)
            v_sb = cb.tile([128, 8], f32)
            nc.scalar.copy(v_sb[:], ps_v[:])

            vh_in = dr.tile([H + HC], f32)
            nc.sync.dma_start(out=vh_in[0:H].rearrange("(c p) -> p c", p=128), in_=v_sb[:])
            nc.sync.dma_start(out=vh_in[H:H + HC].rearrange("(p o) -> p o", o=1), in_=hnew[:])
            vhg = dr.tile([NC, H + HC], f32, addr_space="Shared")
            nc.gpsimd.collective_compute("AllGather", OP.bypass, replica_groups=RG,
                                         ins=[vh_in.opt()], outs=[vhg.opt()])

            vparts = cb.tile([NC, H], f32)
            nc.sync.dma_start(out=vparts[:], in_=vhg[:, 0:H])
            vall = cb.tile([NC, H], f32)
            nc.gpsimd.partition_all_reduce(vall[:], vparts[:], channels=NC,
                                           reduce_op=bass_isa.ReduceOp.add)
            vb = cb.tile([128, H], f32)
            nc.gpsimd.partition_broadcast(vb[:], vall[0:1, :])
            hf = cb.tile([128, 8], f32)
            nc.sync.dma_start(out=hf[:], in_=vhg[:, H:H + HC].rearrange("j p -> p j"))

            # ================= attention scores + local softmax =================
            en = cb.tile([128, 8, H], f32)
            nc.sync.dma_start(out=en[:], in_=encs.ap().rearrange("t p k -> p t k"))
            scv = cb.tile([128, 8], f32)
            for t in range(8):
                prod = scr.tile([128, H], f32, tag="prod")
                nc.vector.tensor_mul(prod[:], en[:, t, :], vb[:])
                nc.vector.tensor_reduce(scv[:, t:t + 1], prod[:],
                                        axis=mybir.AxisListType.X, op=OP.add)

            m1 = cb.tile([128, 1], f32)
            nc.vector.tensor_reduce(m1[:], scv[:], axis=mybir.AxisListType.X, op=OP.max)
            mall = cb.tile([128, 1], f32)
            nc.gpsimd.partition_all_reduce(mall[:], m1[:], channels=128,
                                           reduce_op=bass_isa.ReduceOp.max)
            negm = cb.tile([128, 1], f32)
            nc.vector.tensor_scalar_mul(negm[:], mall[:], -1.0)
            pexp = cb.tile([128, 8], f32)
            s1 = cb.tile([128, 1], f32)
            nc.scalar.activation(pexp[:], scv[:], AF.Exp, bias=negm[:, 0:1], accum_out=s1[:])
            sall = cb.tile([128, 1], f32)
            nc.gpsimd.partition_all_reduce(sall[:], s1[:], channels=128,
                                           reduce_op=bass_isa.ReduceOp.add)

            st = cb.tile([1, 8], f32)
            nc.vector.memset(st[0:1, :], 0.0)
            nc.vector.tensor_copy(st[0:1, 0:1], mall[0:1, :])
            nc.vector.tensor_copy(st[0:1, 1:2], sall[0:1, :])
            st_dr = dr.tile([1, 8], f32)
            nc.sync.dma_start(out=st_dr[:], in_=st[0:1, :])
            statsg = dr.tile([NC, 8], f32, addr_space="Shared")
            nc.gpsimd.collective_compute("AllGather", OP.bypass, replica_groups=RG,
                                         ins=[st_dr.opt()], outs=[statsg.opt()])

            # unnormalized context partial (overlaps the stats AllGather)
            ps_c = ps.tile([128, H], f32, tag="psc")
            for t in range(8):
                for hh in range(2):
                    nc.tensor.matmul(ps_c[0:1, hh * 512:(hh + 1) * 512],
                                     lhsT=pexp[:, t:t + 1],
                                     rhs=en[:, t, hh * 512:(hh + 1) * 512],
                                     start=(t == 0), stop=(t == 7))

            # merge stats -> global (M, Z); this core's scale = exp(m_l - M)/Z
            sst = cb.tile([NC, 2], f32)
            nc.sync.dma_start(out=sst[:], in_=statsg[:, 0:2])
            Mv = cb.tile([NC, 1], f32)
            nc.gpsimd.partition_all_reduce(Mv[:], sst[:, 0:1], channels=NC,
                                           reduce_op=bass_isa.ReduceOp.max)
            negM = cb.tile([NC, 1], f32)
            nc.vector.tensor_scalar_mul(negM[:], Mv[:], -1.0)
            ee = cb.tile([NC, 1], f32)
            nc.scalar.activation(ee[:], sst[:, 0:1], AF.Exp, bias=negM[:, 0:1])
            sz = cb.tile([NC, 1], f32)
            nc.vector.tensor_mul(sz[:], sst[:, 1:2], ee[:])
            Zv = cb.tile([NC, 1], f32)
            nc.gpsimd.partition_all_reduce(Zv[:], sz[:], channels=NC,
                                           reduce_op=bass_isa.ReduceOp.add)
            eloc = cb.tile([1, 1], f32)
            nc.scalar.activation(eloc[:], mall[0:1, :], AF.Exp, bias=negM[0:1, 0:1])
            rzv = cb.tile([1, 1], f32)
            nc.vector.reciprocal(rzv[:], Zv[0:1, :])
            wf = cb.tile([1, 1], f32)
            nc.vector.tensor_mul(wf[:], eloc[:], rzv[:])

            # attention weights output: P * wf
            wfb = cb.tile([128, 1], f32)
            nc.gpsimd.partition_broadcast(wfb[:], wf[0:1, :])
            wl = cb.tile([128, 8], f32)
            nc.vector.tensor_scalar_mul(wl[:], pexp[:], wfb[:, 0:1])
            nc.sync.dma_start(out=attw_part.ap().rearrange("(t p) -> p t", p=128), in_=wl[:])

            # scaled context partial -> AllReduce
            ctx_u = cb.tile([1, H], f32)
            nc.scalar.copy(ctx_u[:], ps_c[0:1, :])
            ctx_s = cb.tile([1, H], f32)
            nc.vector.tensor_scalar_mul(ctx_s[:], ctx_u[:], wf[0:1, 0:1])
            ctxp = dr.tile([H], f32)
            nc.sync.dma_start(out=ctxp[:].rearrange("(o k) -> o k", o=1), in_=ctx_s[:])
            ctxf = dr.tile([H], f32, addr_space="Shared")
            nc.gpsimd.collective_compute("AllReduce", OP.add, replica_groups=RG,
                                         ins=[ctxp.opt()], outs=[ctxf.opt()])
            nc.sync.dma_start(out=ctx_out[:], in_=ctxf[:])
            cfx = cb.tile([128, 8], f32)
            nc.sync.dma_start(out=cfx[:], in_=ctxf[:].rearrange("(j p) -> p j", p=128))

            # bf16 copies of the two rhs-chunk sets
            hcatb = cb.tile([128, 8], bf16)
            nc.vector.tensor_copy(hcatb[:], hf[:])
            ccatb = cb.tile([128, 8], bf16)
            nc.vector.tensor_copy(ccatb[:], cfx[:])

            # ================= logits =================
            lgrow = cb.tile([1, VSP], f32)
            nc.sync.dma_start(out=lgrow[:], in_=obv.ap().rearrange("(o v) -> o v", o=1))
            msl = cb.tile([1, NSL], f32)
            nmsl = cb.tile([1, NSL], f32)
            ssl = cb.tile([1, NSL], f32)

            for half, (wsrc, catt) in enumerate(((wtsh, hcatb), (wtsc, ccatb))):
                for vs in range(NSL):
                    wt = stp.tile([128, 8, 512], bf16, tag="wt")
                    nc.sync.dma_start(out=wt[:], in_=wsrc[vs].rearrange("a b c -> b a c"))
                    psv = psl.tile([1, 512], f32, tag="pslog")
                    for j in range(8):
                        nc.tensor.matmul(psv[0:1, :], lhsT=catt[:, j:j + 1],
                                         rhs=wt[:, j, :], start=(j == 0), stop=(j == 7))
                    sl = lgrow[0:1, vs * 512:(vs + 1) * 512]
                    nc.vector.tensor_add(sl, sl, psv[0:1, :])
                    if half == 1:
                        # per-slice logsumexp stats, overlapped with the stream
                        nc.vector.tensor_reduce(msl[0:1, vs:vs + 1], sl,
                                                axis=mybir.AxisListType.X, op=OP.max)
                        nc.vector.tensor_scalar_mul(nmsl[0:1, vs:vs + 1],
                                                    msl[0:1, vs:vs + 1], -1.0)
                        esc = scr.tile([1, 512], f32, tag="esc")
                        nc.scalar.activation(esc[0:1, :], sl, AF.Exp,
                                             bias=nmsl[0:1, vs:vs + 1],
                                             accum_out=ssl[0:1, vs:vs + 1])

            # merge 13 slice stats -> (m_l, s_l)
            Ml = cb.tile([1, 1], f32)
            nc.vector.tensor_reduce(Ml[:], msl[0:1, :], axis=mybir.AxisListType.X, op=OP.max)
            nMl = cb.tile([1, 1], f32)
            nc.vector.tensor_scalar_mul(nMl[:], Ml[:], -1.0)
            e13 = cb.tile([1, NSL], f32)
            nc.scalar.activation(e13[:], msl[0:1, :], AF.Exp, bias=nMl[0:1, 0:1])
            sz13 = cb.tile([1, NSL], f32)
            nc.vector.tensor_mul(sz13[:], e13[:], ssl[0:1, :])
            sl13 = cb.tile([1, 1], f32)
            nc.vector.tensor_reduce(sl13[:], sz13[0:1, :], axis=mybir.AxisListType.X, op=OP.add)

            st2 = cb.tile([1, 8], f32)
            nc.vector.memset(st2[0:1, :], 0.0)
            nc.vector.tensor_copy(st2[0:1, 0:1], Ml[:])
            nc.vector.tensor_copy(st2[0:1, 1:2], sl13[:])
            st2_dr = dr.tile([1, 8], f32)
            nc.sync.dma_start(out=st2_dr[:], in_=st2[0:1, :])
            stats2g = dr.tile([NC, 8], f32, addr_space="Shared")
            nc.gpsimd.collective_compute("AllGather", OP.bypass, replica_groups=RG,
                                         ins=[st2_dr.opt()], outs=[stats2g.opt()])
            sst2 = cb.tile([NC, 2], f32)
            nc.sync.dma_start(out=sst2[:], in_=stats2g[:, 0:2])
            Mv2 = cb.tile([NC, 1], f32)
            nc.gpsimd.partition_all_reduce(Mv2[:], sst2[:, 0:1], channels=NC,
                                           reduce_op=bass_isa.ReduceOp.max)
            negM2 = cb.tile([NC, 1], f32)
            nc.vector.tensor_scalar_mul(negM2[:], Mv2[:], -1.0)
            ee2 = cb.tile([NC, 1], f32)
            nc.scalar.activation(ee2[:], sst2[:, 0:1], AF.Exp, bias=negM2[:, 0:1])
            sz2 = cb.tile([NC, 1], f32)
            nc.vector.tensor_mul(sz2[:], sst2[:, 1:2], ee2[:])
            Zv2 = cb.tile([NC, 1], f32)
            nc.gpsimd.partition_all_reduce(Zv2[:], sz2[:], channels=NC,
                                           reduce_op=bass_isa.ReduceOp.add)
            lnz = cb.tile([1, 1], f32)
            nc.scalar.activation(lnz[:], Zv2[0:1, :], AF.Ln)
            bsum = cb.tile([1, 1], f32)
            nc.vector.tensor_add(bsum[:], lnz[:], Mv2[0:1, :])
            negb = cb.tile([1, 1], f32)
            nc.vector.tensor_scalar_mul(negb[:], bsum[:], -1.0)
            nc.vector.tensor_scalar_add(lgrow[0:1, :], lgrow[0:1, :], negb[0:1, 0:1])
            nc.sync.dma_start(out=out_part.ap().rearrange("(o v) -> o v", o=1),
                              in_=lgrow[0:1, :])

    nc.compile()
    _cache["nc"] = nc
    return nc


def _prep(word_input, last_context, last_hidden, encoder_outputs,
          emb, W_ih, W_hh, b_ih, b_hh, attn_W, attn_b, out_W, out_b):
    """Host-side sharding: build the 8 per-core input maps."""
    from concourse import mybir
    bf16 = mybir.dt.np(mybir.dt.bfloat16)

    word_input = np.asarray(word_input)
    last_context = np.asarray(last_context, np.float32)
    last_hidden = np.asarray(last_hidden, np.float32)
    enc = np.asarray(encoder_outputs, np.float32)[:, 0, :]       # [S, H]
    emb = np.asarray(emb, np.float32)
    W_ih = np.asarray(W_ih, np.float32)
    W_hh = np.asarray(W_hh, np.float32)
    b_ih = np.asarray(b_ih, np.float32)
    b_hh = np.asarray(b_hh, np.float32)
    attn_W = np.asarray(attn_W, np.float32)
    out_W = np.asarray(out_W, np.float32)
    out_b = np.asarray(out_b, np.float32)

    w = int(word_input.reshape(-1)[0])
    xin = np.ascontiguousarray(np.concatenate([emb[w], last_context[0]]))  # [2048]
    hin = np.ascontiguousarray(last_hidden[0, 0])                          # [1024]

    # out_W: pad vocab to VP, transpose to [core, vs, kc, p, v] in bf16
    owp = np.zeros((VP, K2), np.float32)
    owp[:V] = out_W
    wts = np.ascontiguousarray(
        owp.reshape(NC, NSL, 512, 16, 128).transpose(0, 1, 3, 4, 2)).astype(bf16)
    obp = np.full((VP,), -1e30, np.float32)
    obp[:V] = out_b

    in_maps = []
    for i in range(NC):
        hc = slice(128 * i, 128 * (i + 1))
        rows = np.r_[np.arange(128 * i, 128 * (i + 1)),
                     np.arange(1024 + 128 * i, 1024 + 128 * (i + 1)),
                     np.arange(2048 + 128 * i, 2048 + 128 * (i + 1))]
        wgt = np.ascontiguousarray(W_ih[rows].T.reshape(16, 128, 384))
        wht = np.ascontiguousarray(W_hh[rows].T.reshape(8, 128, 384))
        bias = np.stack([(b_ih + b_hh)[128 * i:128 * (i + 1)],
                         (b_ih + b_hh)[1024 + 128 * i:1024 + 128 * (i + 1)],
                         b_ih[2048 + 128 * i:2048 + 128 * (i + 1)],
                         b_hh[2048 + 128 * i:2048 + 128 * (i + 1)]], axis=1)
        in_maps.append({
            "xin": xin,
            "hin": hin,
            "hch": np.ascontiguousarray(hin[hc]),
            "wgt": wgt,
            "wht": wht,
            "biasv": np.ascontiguousarray(bias.astype(np.float32)),
            "attAr": np.ascontiguousarray(attn_W[hc, :]),
            "encs": np.ascontiguousarray(
                enc[SC * i:SC * (i + 1)].reshape(8, 128, H)),
            "wtsh": np.ascontiguousarray(wts[i, :, 0:8]),
            "wtsc": np.ascontiguousarray(wts[i, :, 8:16]),
            "obv": np.ascontiguousarray(obp[VSP * i:VSP * (i + 1)]),
        })
    return in_maps


def _assemble(results):
    out = np.concatenate([results[i]["out_part"] for i in range(NC)])[:V]
    output = out.reshape(1, V).astype(np.float32)
    context = results[0]["ctx_out"].reshape(1, H).astype(np.float32)
    hidden = np.concatenate(
        [results[i]["hid_part"] for i in range(NC)]).reshape(1, 1, H).astype(np.float32)
    attw = np.concatenate(
        [results[i]["attw_part"] for i in range(NC)]).reshape(1, 1, S).astype(np.float32)
    return output, context, hidden, attw


def run(inputs, trace=False):
    from concourse.bass_utils import run_bass_kernel_spmd
    nc = _build()
    in_maps = _prep(**inputs)
    res = run_bass_kernel_spmd(nc, in_maps, list(range(NC)), trace=trace)
    return _assemble(res.results), res


def kernel(**inputs):
    (output, context, hidden, attw), _ = run(inputs, trace=False)
    return output, context, hidden, attw
